# revision 13
# baseline (speedup 1.0000x reference)
"""Trainium2 Bass kernel for a GQA attention block (dense_transformer).

Reference computation (per core c of 8, tensor-parallel over heads):
  q = x @ wq[:, 256c:256c+256]   -> 2 query heads of dim 128
  k = x @ wk[:, 128g:128g+128]   -> 1 kv head (g = c//2, shared by 2 cores)
  v = x @ wv[:, 128g:128g+128]
  RoPE on q, k; causal softmax attention; o = attn @ v
  out_partial = o @ wo[256c:256c+256, :]     (full [4096, 2048] partial sum)
Host sums the 8 partials.

Device layout: everything transposed ([dim, seq]) so matmuls need no
on-chip transposes of activations:
  QT/KT:  [128 d, 4096 s]  (from projections; RoPE applied on evacuation)
  scores^T[k, q] = KT_blk.T @ QT  (lhsT=KT block, rhs=QT columns)
  P^T = exp(scores^T) (ACT, scale=1/sqrt(128)); causal via 0/1 bf16 mask mul
  O^T += V_blk.T @ P^T            (lhsT=V block [s,d], rhs=P^T)
  row sums via ones-matmul into psum; normalization via K=1 broadcast matmul
  out[s, dm] = O^T_blk.T @ wo     (lhsT=O^T block, rhs=wo rows)
"""

import os
import numpy as np
import ml_dtypes

S = 4096
DM = 2048
HD = 128
NCORES = 8
QSUP = 512          # query supertile (free dim of scores^T psum)
NT = S // QSUP      # 8
NKB = S // 128      # 32 key blocks
SCALE = float(1.0 / np.sqrt(HD))
THETA = 10000.0

_CACHE = {}


def _tctile(tc, shape, dtype, name):
    return tc.nc.alloc_sbuf_tensor(name, list(shape), dtype).ap()



def _build_nc(loop_iters=1):
    import contextlib
    import concourse.mybir as mybir
    import concourse.tile as tile
    from concourse import bacc
    from concourse.masks import make_identity

    dt = mybir.dt
    f32 = dt.float32
    bf16 = dt.bfloat16
    AF = mybir.ActivationFunctionType

    nc = bacc.Bacc("TRN2", target_bir_lowering=False, debug=False, num_devices=NCORES)

    xT_d = nc.dram_tensor("xT", [DM, S], bf16, kind="ExternalInput")
    wq_d = nc.dram_tensor("wq", [DM, 256], bf16, kind="ExternalInput")
    wk_d = nc.dram_tensor("wk", [DM, 128], bf16, kind="ExternalInput")
    wv_d = nc.dram_tensor("wv", [DM, 128], bf16, kind="ExternalInput")
    wo_d = nc.dram_tensor("wo", [256, DM], bf16, kind="ExternalInput")
    cosT_d = nc.dram_tensor("cosT", [HD, S], bf16, kind="ExternalInput")
    sinT_d = nc.dram_tensor("sinT", [HD, S], bf16, kind="ExternalInput")
    maskB_d = nc.dram_tensor("maskB", [128, 2 * QSUP], bf16, kind="ExternalInput")
    out_d = nc.dram_tensor("out", [S, DM], dt.float16, kind="ExternalOutput")

    xT = xT_d.ap()
    wo = wo_d.ap()
    out = out_d.ap()

    with tile.TileContext(nc) as tc:
        # ---- persistent sbuf tensors ----
        QT0 = _tctile(tc, [128, S], bf16, name="QT0")
        QT1 = _tctile(tc, [128, S], bf16, name="QT1")
        KT = _tctile(tc, [128, S], bf16, name="KT")
        VT = _tctile(tc, [128, S], bf16, name="VT")    # [d, s] pre-transpose
        V = _tctile(tc, [128, S], bf16, name="V")      # [s, d] blocks at cols 128*kb
        OT0 = _tctile(tc, [128, S], bf16, name="OT0")
        OT1 = _tctile(tc, [128, S], bf16, name="OT1")
        wq_sb = _tctile(tc, [128, 16 * 256], bf16, name="wq_sb")
        wk_sb = _tctile(tc, [128, 16 * 128], bf16, name="wk_sb")
        wv_sb = _tctile(tc, [128, 16 * 128], bf16, name="wv_sb")
        wo0_sb = _tctile(tc, [128, DM], bf16, name="wo0_sb")
        wo1_sb = _tctile(tc, [128, DM], bf16, name="wo1_sb")
        maskB = _tctile(tc, [128, 2 * QSUP], bf16, name="maskB_sb")
        ident = _tctile(tc, [128, 128], bf16, name="ident")
        ones2 = _tctile(tc, [128, 1], bf16, name="ones2")
        ones1 = _tctile(tc, [1, 128], f32, name="ones1")

        # constants
        make_identity(nc, ident[:, :])
        nc.gpsimd.memset(ones2[:, :], 1.0)
        nc.gpsimd.memset(ones1[:, :], 1.0)

        # weight loads
        for kc in range(16):
            nc.sync.dma_start(wq_sb[:, 256 * kc:256 * kc + 256],
                              wq_d.ap()[128 * kc:128 * kc + 128, :])
            nc.sync.dma_start(wk_sb[:, 128 * kc:128 * kc + 128],
                              wk_d.ap()[128 * kc:128 * kc + 128, :])
            nc.sync.dma_start(wv_sb[:, 128 * kc:128 * kc + 128],
                              wv_d.ap()[128 * kc:128 * kc + 128, :])
        nc.sync.dma_start(wo0_sb[:, :], wo[0:128, :])
        nc.sync.dma_start(wo1_sb[:, :], wo[128:256, :])
        nc.sync.dma_start(maskB[:, :], maskB_d.ap()[:, :])

        loop_ctx = (tc.For_i(0, loop_iters, 1) if loop_iters > 1
                    else contextlib.nullcontext())
        with loop_ctx:
            _emit_body(nc, tc, mybir, locals())

    nc.compile()
    return nc


def _emit_body(nc, tc, mybir, env):
    dt = mybir.dt
    f32 = dt.float32
    bf16 = dt.bfloat16
    AF = mybir.ActivationFunctionType
    xT = env["xT"]
    wo = env["wo"]
    out = env["out"]
    cosT_d = env["cosT_d"]
    sinT_d = env["sinT_d"]
    QT0, QT1, KT, VT, V, OT0, OT1 = (env[k] for k in
                                     ("QT0", "QT1", "KT", "VT", "V", "OT0", "OT1"))
    wq_sb, wk_sb, wv_sb, wo0_sb, wo1_sb = (env[k] for k in
                                           ("wq_sb", "wk_sb", "wv_sb",
                                            "wo0_sb", "wo1_sb"))
    maskB, ident, ones2, ones1 = (env[k] for k in
                                  ("maskB", "ident", "ones2", "ones1"))

    if True:
        # ================= phase 1: projections + RoPE =================
        with tc.tile_pool(name="p1sb", bufs=2) as p1sb, \
             tc.tile_pool(name="p1ps", bufs=2, space="PSUM") as p1ps, \
             tc.tile_pool(name="p1rp", bufs=3) as p1rp:
            for sc in range(8):
                cs = slice(512 * sc, 512 * sc + 512)
                xt = p1sb.tile([128, 16 * 512], bf16, tag="xt")
                for kc in range(16):
                    nc.sync.dma_start(xt[:, 512 * kc:512 * kc + 512],
                                      xT[128 * kc:128 * kc + 128, cs])
                cos_t = p1sb.tile([128, 512], bf16, tag="cos")
                sin_t = p1sb.tile([128, 512], bf16, tag="sin")
                nc.sync.dma_start(cos_t[:, :], cosT_d.ap()[:, cs])
                nc.sync.dma_start(sin_t[:, :], sinT_d.ap()[:, cs])

                def proj(w_sb, wstride, hofs, ps_tag, name):
                    ps = p1ps.tile([128, 512], f32, tag=ps_tag, name=name)
                    for kc in range(16):
                        nc.tensor.matmul(
                            ps[:, :],
                            w_sb[:, wstride * kc + hofs:wstride * kc + hofs + 128],
                            xt[:, 512 * kc:512 * kc + 512],
                            start=(kc == 0), stop=(kc == 15))
                    return ps

                def rope(ps, dst, name):
                    # q' = q * cos + swap64(q) * sin_folded
                    qraw = p1rp.tile([128, 512], bf16, tag="qraw",
                                     name=f"qraw_{name}")
                    nc.scalar.copy(qraw[:, :], ps[:, :])
                    qsw = p1rp.tile([128, 512], bf16, tag="qsw",
                                    name=f"qsw_{name}")
                    nc.sync.dma_start(qsw[0:64, :], qraw[64:128, :])
                    nc.sync.dma_start(qsw[64:128, :], qraw[0:64, :])
                    m1 = p1rp.tile([128, 512], bf16, tag="m1", name=f"m1_{name}")
                    nc.vector.tensor_mul(m1[:, :], qraw[:, :], cos_t[:, :])
                    nc.vector.tensor_mul(qsw[:, :], qsw[:, :], sin_t[:, :])
                    nc.vector.tensor_add(dst[:, cs], m1[:, :], qsw[:, :])

                rope(proj(wq_sb, 256, 0, "psq", f"psq0_{sc}"), QT0, f"q0_{sc}")
                rope(proj(wq_sb, 256, 128, "psq", f"psq1_{sc}"), QT1, f"q1_{sc}")
                rope(proj(wk_sb, 128, 0, "psk", f"psk_{sc}"), KT, f"k_{sc}")
                psv = proj(wv_sb, 128, 0, "psv", f"psv_{sc}")
                nc.vector.tensor_copy(VT[:, cs], psv[:, :])

        # V transpose: VT [d, s] -> V [s, d] per 128-block
        with tc.tile_pool(name="vtps", bufs=2, space="PSUM") as vtps:
            for kb in range(NKB):
                bs = slice(128 * kb, 128 * kb + 128)
                tp = vtps.tile([128, 128], bf16, tag="vtp")
                nc.tensor.transpose(tp[:, :], VT[:, bs], ident[:, :])
                nc.vector.tensor_copy(V[:, bs], tp[:, :])

        # ================= phase 2: attention + out-proj =================
        with tc.tile_pool(name="scps", bufs=2, space="PSUM") as scps, \
             tc.tile_pool(name="oaps", bufs=1, space="PSUM") as oaps, \
             tc.tile_pool(name="smps", bufs=1, space="PSUM") as smps, \
             tc.tile_pool(name="opps", bufs=2, space="PSUM") as opps, \
             tc.tile_pool(name="ptpool", bufs=3) as ptpool, \
             tc.tile_pool(name="nrm", bufs=2) as nrm, \
             tc.tile_pool(name="outsb", bufs=3) as outsb:
            for t in range(NT):
                qs = slice(QSUP * t, QSUP * t + QSUP)
                nkb = 4 * t + 4
                oacc = [oaps.tile([128, QSUP], f32, tag="oacc0", name=f"oacc0_{t}"),
                        oaps.tile([128, QSUP], f32, tag="oacc1", name=f"oacc1_{t}")]
                sums = [smps.tile([1, QSUP], f32, tag="sums0", name=f"sums0_{t}"),
                        smps.tile([1, QSUP], f32, tag="sums1", name=f"sums1_{t}")]
                QTs = [QT0, QT1]
                for kb in range(nkb):
                    bs = slice(128 * kb, 128 * kb + 128)
                    j = kb - 4 * t
                    for h in range(2):
                        sc_ps = scps.tile([128, QSUP], f32, tag="sc",
                                          name=f"sc_{t}_{kb}_{h}")
                        nc.tensor.matmul(sc_ps[:, :], KT[:, bs], QTs[h][:, qs],
                                         start=True, stop=True)
                        pt = ptpool.tile([128, QSUP], bf16, tag="pt",
                                         name=f"pt_{t}_{kb}_{h}")
                        nc.scalar.activation(pt[:, :], sc_ps[:, :], AF.Exp,
                                             scale=SCALE)
                        if 0 <= j:
                            nc.vector.tensor_mul(
                                pt[:, :], pt[:, :],
                                maskB[:, QSUP - 128 * j:2 * QSUP - 128 * j])
                        nc.tensor.matmul(oacc[h][:, :], V[:, bs], pt[:, :],
                                         start=(kb == 0), stop=(kb == nkb - 1))
                        nc.tensor.matmul(sums[h][:, :], ones2[:, :], pt[:, :],
                                         start=(kb == 0), stop=(kb == nkb - 1))
                # normalize
                OTs = [OT0, OT1]
                for h in range(2):
                    rs = nrm.tile([1, QSUP], f32, tag="rs", name=f"rs_{t}_{h}")
                    nc.vector.reciprocal(rs[:, :], sums[h][:, :])
                    r_ps = scps.tile([128, QSUP], f32, tag="sc", name=f"rps_{t}_{h}")
                    nc.tensor.matmul(r_ps[:, :], ones1[:, :], rs[:, :],
                                     start=True, stop=True)
                    r_sb = nrm.tile([128, QSUP], f32, tag="rsb", name=f"rsb_{t}_{h}")
                    nc.scalar.copy(r_sb[:, :], r_ps[:, :])
                    nc.vector.tensor_mul(OTs[h][:, qs], oacc[h][:, :], r_sb[:, :])
                # out-projection for the 4 s-subtiles of this supertile
                for sst in range(4):
                    st = 4 * t + sst
                    ss = slice(128 * st, 128 * st + 128)
                    for ncol in range(4):
                        o_ps = opps.tile([128, 512], f32, tag="op",
                                         name=f"op_{st}_{ncol}")
                        nc.tensor.matmul(o_ps[:, :], OT0[:, ss],
                                         wo0_sb[:, 512 * ncol:512 * ncol + 512],
                                         start=True, stop=False)
                        nc.tensor.matmul(o_ps[:, :], OT1[:, ss],
                                         wo1_sb[:, 512 * ncol:512 * ncol + 512],
                                         start=False, stop=True)
                        ob = outsb.tile([128, 512], f32, tag="ob",
                                        name=f"ob_{st}_{ncol}")
                        nc.vector.tensor_copy(ob[:, :], o_ps[:, :])
                        nc.sync.dma_start(
                            out[ss, 512 * ncol:512 * ncol + 512], ob[:, :])


def _host_prep(x, wq, wk, wv, wo):
    bf16 = ml_dtypes.bfloat16
    xT = np.ascontiguousarray(np.asarray(x, np.float32)[0].T).astype(bf16)

    inv_freq = 1.0 / (THETA ** (np.arange(0, HD, 2, np.float32) / HD))
    pos = np.arange(S, dtype=np.float32)
    freqs = pos[:, None] * inv_freq[None, :]
    emb = np.concatenate([freqs, freqs], axis=-1)      # [S, 128]
    cosT = np.ascontiguousarray(np.cos(emb).T).astype(bf16)
    # sign-folded sin table: rows 0:64 negated (q' = q*cos + swap64(q)*sinT)
    sinT_f = np.sin(emb).T.copy()
    sinT_f[0:64, :] *= -1.0
    sinT = np.ascontiguousarray(sinT_f).astype(bf16)

    kk = np.arange(128)[:, None]
    cc = np.arange(2 * QSUP)[None, :]
    maskB = (cc >= kk + QSUP).astype(bf16)

    wq = np.asarray(wq, np.float32)
    wk = np.asarray(wk, np.float32)
    wv = np.asarray(wv, np.float32)
    wo = np.asarray(wo, np.float32)

    in_maps = []
    for c in range(NCORES):
        g = c // 2
        in_maps.append({
            "xT": xT,
            "wq": np.ascontiguousarray(wq[:, 256 * c:256 * c + 256]).astype(bf16),
            "wk": np.ascontiguousarray(wk[:, 128 * g:128 * g + 128]).astype(bf16),
            "wv": np.ascontiguousarray(wv[:, 128 * g:128 * g + 128]).astype(bf16),
            "wo": np.ascontiguousarray(wo[256 * c:256 * c + 256, :]).astype(bf16),
            "cosT": cosT,
            "sinT": sinT,
            "maskB": maskB,
        })
    return in_maps


def get_nc():
    if "nc" not in _CACHE:
        _CACHE["nc"] = _build_nc()
    return _CACHE["nc"]


def kernel(x, wq, wk, wv, wo):
    from concourse.bass_utils import run_bass_kernel_spmd

    nc = get_nc()
    in_maps = _host_prep(x, wq, wk, wv, wo)
    res = run_bass_kernel_spmd(nc, in_maps, core_ids=list(range(NCORES)))
    _CACHE["last_results"] = res
    acc = res.results[0]["out"].astype(np.float32)
    for c in range(1, NCORES):
        acc = acc + res.results[c]["out"]
    return acc.reshape(1, S, DM)
